# revision 20
# baseline (speedup 1.0000x reference)
"""MsPoE Llama attention on 8 TRN2 NeuronCores (tensor-parallel over heads).

Strategy (v2, fused single launch)
----------------------------------
The Ms-PoE head ordering depends only on the LAST row of the first
softmax, which depends only on hs/Wq/Wk — all known on the host before
any launch. So the stats path runs on the HOST (fp64, margins ~3e-5 vs
the reference's fp32), and the device does ONE fused launch per core:

  per core (4 heads):
    S1: Q/K projections (bf16, W-stationary) -> per-head RoPE fused into
        the PSUM drain path (DVE + swap-half SBUF DMAs)
    S2: V projection (hs-stationary -> natural [s, d] layout)
    S3: causal attention per (head, 512-query block): scoresT = k^T q per
        128-key tile, exp on ACT (bf16 out), Z via elementwise ext
        accumulation (DVE/Pool) + one ones-matmul per block,
        normalization fused in-flight via 1/Z DRAM-broadcast roundtrip
    S4: o_proj partial oT[e, s] (bf16 out)

  host: argsort stats, permuted per-head RoPE cos/sin caches (bf16),
        final 8-way partial sum in fp64.

All matmuls are bf16 (PE full rate); PSUM accumulates fp32. Total
device rel err ~5e-3 against the fp32 reference, well under the 2e-2
gate. q/k/v never round-trip through HBM (SBUF-resident bf16).
"""

import os
import sys

import numpy as np

for _p in ("/opt/trn_rl_repo", "/root/.axon_site/_ro/trn_rl_repo"):
    if os.path.isdir(_p) and _p not in sys.path:
        sys.path.append(_p)

import ml_dtypes  # noqa: E402

import concourse.bass as bass  # noqa: E402
import concourse.tile as tile  # noqa: E402
from concourse import bacc, mybir  # noqa: E402
from concourse import bass_utils  # noqa: E402

F32 = mybir.dt.float32
F32R = mybir.dt.float32r
BF16 = mybir.dt.bfloat16
BF_NP = ml_dtypes.bfloat16

B, S, HID, H, D = 1, 2048, 4096, 32, 128
NCORES, HPC = 8, 4          # cores, heads per core
JC = HPC * D                # 512: per-core projection width
KT = HID // 128             # 32 contraction tiles
SB = S // 512               # 4 sequence blocks
ST = S // 128               # 16 key 128-tiles
BASE, MIN_R, MAX_R = 10000.0, 1.0, 3.0
SCALE = 1.0 / float(np.sqrt(D))
NEGM = -1.0e35              # additive causal mask value (exp -> 0)

_CACHE = {}
_LAST_RES = None
DEBUG = False
TRACE = False          # set True (e.g. from test.py) to profile the launch
LAST_PROFILE = {}      # filled with BassKernelResults when TRACE is on


def build():
    nc = bacc.Bacc("TRN2", target_bir_lowering=False, debug=False, num_devices=NCORES)
    hsT = nc.dram_tensor("hsT", [HID, S], BF16, kind="ExternalInput").ap()
    wqT = nc.dram_tensor("wqT", [HID, JC], BF16, kind="ExternalInput").ap()
    wkT = nc.dram_tensor("wkT", [HID, JC], BF16, kind="ExternalInput").ap()
    wvT = nc.dram_tensor("wvT", [HID, JC], BF16, kind="ExternalInput").ap()
    woT = nc.dram_tensor("woT", [JC, HID], BF16, kind="ExternalInput").ap()
    cosT = nc.dram_tensor("cosT", [JC, S], BF16, kind="ExternalInput").ap()
    shatT = nc.dram_tensor("shatT", [JC, S], BF16, kind="ExternalInput").ap()
    masks = nc.dram_tensor("masks", [128, 4 * 512], BF16, kind="ExternalInput").ap()
    oT = nc.dram_tensor("oT", [HID, S], BF16, kind="ExternalOutput").ap()
    if DEBUG:
        d_rq = nc.dram_tensor("d_rq", [128, HPC * S], BF16, kind="ExternalOutput").ap()
        d_rk = nc.dram_tensor("d_rk", [128, HPC * S], BF16, kind="ExternalOutput").ap()
        d_v = nc.dram_tensor("d_v", [128, ST * JC], BF16, kind="ExternalOutput").ap()
        d_at = nc.dram_tensor("d_at", [128, HPC * SB * 512], BF16, kind="ExternalOutput").ap()
        d_z = nc.dram_tensor("d_z", [HPC * SB, 512], F32, kind="ExternalOutput").ap()

    hsT_b = hsT.rearrange("(kt p) s -> p kt s", p=128)      # [128, 32, S]
    wqT_b = wqT.rearrange("(kt p) j -> p kt j", p=128)      # [128, 32, JC]
    wkT_b = wkT.rearrange("(kt p) j -> p kt j", p=128)
    wvT_b = wvT.rearrange("(kt p) j -> p kt j", p=128)
    woT_b = woT.rearrange("(jt p) e -> p jt e", p=128)      # [128, 4, HID]
    cosT_b = cosT.rearrange("(h p) s -> p h s", p=128)      # [128, 4, S]
    shatT_b = shatT.rearrange("(h p) s -> p h s", p=128)
    oT_b = oT.rearrange("(et p) s -> p et s", p=128)        # [128, 32, S]

    with tile.TileContext(nc) as tc:
        with (
            tc.tile_pool(name="wbig", bufs=3) as wbig,        # Wq/Wk/Wv/Wo ring
            tc.tile_pool(name="hpool", bufs=7) as hpool,      # hs stream [128,512]
            tc.tile_pool(name="qkch", bufs=4) as qkch,        # raw q/k chunks
            tc.tile_pool(name="swp", bufs=2) as swp,          # swapped-half chunks
            tc.tile_pool(name="csld", bufs=3) as csld,        # cos/sin chunks
            tc.tile_pool(name="rtmp", bufs=2) as rtmp,        # rope temps
            tc.tile_pool(name="ropes", bufs=1) as ropes,      # rq/rk resident
            tc.tile_pool(name="vres", bufs=1) as vres,        # v natural resident
            tc.tile_pool(name="attn", bufs=1) as attnp,       # attnT resident
            tc.tile_pool(name="expp", bufs=3) as expp,        # exp tiles bf16
            tc.tile_pool(name="accp", bufs=3) as accp,        # Z accumulators f32r
            tc.tile_pool(name="zp", bufs=2) as zp,            # z rows / broadcasts
            tc.tile_pool(name="small", bufs=1) as small,
            tc.tile_pool(name="outp", bufs=2) as outp,        # o_proj staging
            tc.tile_pool(name="dramp", bufs=1, space="DRAM") as dramp,
            tc.tile_pool(name="ps", bufs=8, space="PSUM") as ps,
        ):
            # ---------------- constants / weight loads ----------------
            masks_sb = small.tile([128, 4, 512], BF16)
            onesf_c = small.tile([128, 1], F32)
            nc.vector.memset(onesf_c, 1.0)
            ones_col = small.tile([128, 1], F32R)
            nc.vector.tensor_copy(ones_col, onesf_c)

            wq_sb = wbig.tile([128, KT, JC], BF16, tag="w", name="wq")
            wk_sb = wbig.tile([128, KT, JC], BF16, tag="w", name="wk")
            wv_sb = wbig.tile([128, KT, JC], BF16, tag="w", name="wv")

            rq = ropes.tile([128, HPC, S], BF16, name="rq")
            rk = ropes.tile([128, HPC, S], BF16, name="rk")
            vnat = vres.tile([128, ST, JC], BF16, name="vnat")
            attnT = attnp.tile([128, HPC * SB, 512], BF16, name="attnT")
            zd = dramp.tile([HPC * SB, 512], F32)

            def rope_chunk(h, sb, qch, kch):
                """rq/rk[:, h, sb*512:...] from raw chunks qch/kch [128,512]."""
                ss = slice(sb * 512, (sb + 1) * 512)
                cch = csld.tile([128, 512], BF16, tag="cs", name=f"c{h}_{sb}")
                nc.sync.dma_start(cch, cosT_b[:, h, ss])
                sch = csld.tile([128, 512], BF16, tag="cs", name=f"s{h}_{sb}")
                nc.sync.dma_start(sch, shatT_b[:, h, ss])
                qsw = swp.tile([128, 512], BF16, tag="sw", name=f"qsw{h}_{sb}")
                nc.sync.dma_start(qsw[0:64, :], qch[64:128, :])
                nc.sync.dma_start(qsw[64:128, :], qch[0:64, :])
                ksw = swp.tile([128, 512], BF16, tag="sw", name=f"ksw{h}_{sb}")
                nc.sync.dma_start(ksw[0:64, :], kch[64:128, :])
                nc.sync.dma_start(ksw[64:128, :], kch[0:64, :])
                with nc.allow_low_precision(reason="rope bf16"):
                    for eng, dst, raw, sw in (
                        (nc.vector, rq, qch, qsw),
                        (nc.gpsimd, rk, kch, ksw),
                    ):
                        d = dst[:, h, ss]
                        t2 = rtmp.tile([128, 512], BF16, tag="rt")
                        eng.tensor_mul(d, raw, cch)
                        eng.tensor_mul(t2, sw, sch)
                        eng.tensor_add(d, d, t2)

            def s1_qk_block(sb):
                ss = slice(sb * 512, (sb + 1) * 512)
                ps_q = [ps.tile([128, 512], F32, tag="ps", name=f"pq{sb}_{j}")
                        for j in range(HPC)]
                ps_k = [ps.tile([128, 512], F32, tag="ps", name=f"pk{sb}_{j}")
                        for j in range(HPC)]
                for kt in range(KT):
                    if sb == 0:
                        k1 = slice(kt, kt + 1)
                        nc.sync.dma_start(wq_sb[:, k1], wqT_b[:, k1])
                        nc.sync.dma_start(wk_sb[:, k1], wkT_b[:, k1])
                    if sb in (1, 2) and kt % 8 == 0:
                        # paced wv load (0.5MB per 8 kt) hides in the stream
                        ck = (sb - 1) * 4 + kt // 8
                        c4 = slice(ck * 4, (ck + 1) * 4)
                        nc.sync.dma_start(wv_sb[:, c4], wvT_b[:, c4])
                    if sb == 3 and kt == 0:
                        nc.sync.dma_start(
                            masks_sb, masks.rearrange("p (r j) -> p r j", r=4)
                        )
                    hst = hpool.tile([128, 512], BF16, tag="h",
                                     name=f"h1_{sb}_{kt}")
                    nc.sync.dma_start(hst, hsT_b[:, kt, ss])
                    for jt in range(HPC):
                        js = slice(jt * 128, (jt + 1) * 128)
                        nc.tensor.matmul(
                            ps_q[jt], wq_sb[:, kt, js], hst,
                            start=(kt == 0), stop=(kt == KT - 1),
                        )
                        nc.tensor.matmul(
                            ps_k[jt], wk_sb[:, kt, js], hst,
                            start=(kt == 0), stop=(kt == KT - 1),
                        )
                qcs, kcs = [], []
                for jt in range(HPC):
                    qcj = qkch.tile([128, 512], BF16, tag="qc", name=f"qc{sb}_{jt}")
                    kcj = qkch.tile([128, 512], BF16, tag="kc", name=f"kc{sb}_{jt}")
                    nc.scalar.copy(qcj, ps_q[jt])
                    nc.vector.tensor_copy(kcj, ps_k[jt])
                    qcs.append(qcj)
                    kcs.append(kcj)
                for h in range(HPC):
                    rope_chunk(h, sb, qcs[h], kcs[h])

            def s2_v_block(sb, wo_sb):
                ss = slice(sb * 512, (sb + 1) * 512)
                ps_v = [ps.tile([128, 512], F32, tag="ps", name=f"pv{sb}_{t}")
                        for t in range(4)]
                for kt in range(KT):
                    if kt % 16 == 8:
                        # paced wo load (0.5MB per 16 kt) for the S3 filler
                        ck = sb * 2 + kt // 16
                        cw = slice(ck * 512, (ck + 1) * 512)
                        nc.sync.dma_start(wo_sb[:, :, cw], woT_b[:, :, cw])
                    hst = hpool.tile([128, 512], BF16, tag="h", name=f"h2_{sb}_{kt}")
                    nc.sync.dma_start(hst, hsT_b[:, kt, ss])
                    for t4 in range(4):
                        cs = slice(t4 * 128, (t4 + 1) * 128)
                        nc.tensor.matmul(
                            ps_v[t4], hst[:, cs], wv_sb[:, kt],
                            start=(kt == 0), stop=(kt == KT - 1),
                        )
                for t4 in range(4):
                    if t4 % 2 == 0:
                        nc.scalar.copy(vnat[:, sb * 4 + t4], ps_v[t4])
                    else:
                        nc.vector.tensor_copy(vnat[:, sb * 4 + t4], ps_v[t4])

            pending_z = []
            lag = [1]

            def flush_z():
                # deferred Z chain: the ones-matmul waits on the Pool ext-sum,
                # so it is issued AFTER the next block's matmuls to avoid
                # head-of-line blocking the in-order PE queue
                while len(pending_z) > lag[0]:
                    i16, acc = pending_z.pop(0)
                    ps_z = ps.tile([1, 512], F32, tag="ps", name=f"pz{i16}")
                    nc.tensor.matmul(ps_z, ones_col, acc, start=True, stop=True)
                    zr1 = zp.tile([1, 512], F32, tag="zr", name=f"zr{i16}")
                    nc.vector.reciprocal(zr1, ps_z)
                    nc.sync.dma_start(zd[i16:i16 + 1, :], zr1)
                    zb = zp.tile([128, 512], F32, tag="zb", name=f"zb{i16}",
                                 bufs=3)
                    nc.sync.dma_start(
                        zb, zd[i16:i16 + 1, :].to_broadcast((128, 512))
                    )
                    with nc.allow_low_precision(reason="attn normalize bf16"):
                        nc.gpsimd.tensor_mul(attnT[:, i16], attnT[:, i16], zb)

            def s3_attn_block(h, qb, filler):
                hs_ = slice(h * 128, (h + 1) * 128)
                nkt = 4 * qb + 4
                qs = slice(qb * 512, (qb + 1) * 512)
                i16 = h * SB + qb
                ps_o = ps.tile([128, 512], F32, tag="ps", name=f"po{i16}")
                acc = accp.tile([128, 512], F32R, tag="acc", name=f"acc{i16}")
                for kt in range(nkt):
                    ps_s = ps.tile([128, 512], F32, tag="ps",
                                   name=f"s{i16}_{kt}")
                    nc.tensor.matmul(
                        ps_s, rk[:, h, kt * 128:(kt + 1) * 128], rq[:, h, qs],
                        start=True, stop=True,
                    )
                    r = kt - 4 * qb
                    if r >= 0:
                        nc.vector.tensor_add(ps_s, ps_s, masks_sb[:, r])
                    ext = expp.tile([128, 512], BF16, tag="exp")
                    nc.scalar.activation(
                        ext, ps_s, mybir.ActivationFunctionType.Exp,
                        scale=SCALE,
                    )
                    with nc.allow_low_precision(reason="z accum f32r"):
                        if kt == 0:
                            nc.gpsimd.tensor_copy(acc, ext)
                        else:
                            nc.gpsimd.tensor_add(acc, acc, ext)
                    nc.tensor.matmul(
                        ps_o, vnat[:, kt, hs_], ext,
                        start=(kt == 0), stop=(kt == nkt - 1),
                    )
                    for _ in range(2):
                        if filler:
                            filler.pop(0)()
                flush_z()
                with nc.allow_low_precision(reason="attn drain bf16"):
                    nc.vector.tensor_copy(attnT[:, i16], ps_o)
                pending_z.append((i16, acc))

            def oproj_ops(sb):
                # per-et closures: emitted one at a time as always-ready PE
                # filler between attention kt iterations
                def mk(et):
                    def op():
                        es = slice(et * 128, (et + 1) * 128)
                        oo = outp.tile([128, 512], BF16, tag="oo",
                                       name=f"oo{et}_{sb}")
                        ps_oo = ps.tile([128, 512], F32, tag="ps",
                                        name=f"poo{et}_{sb}")
                        for jt in range(HPC):
                            nc.tensor.matmul(
                                ps_oo, wo_sb[:, jt, es],
                                attnT[:, jt * SB + sb],
                                start=(jt == 0), stop=(jt == HPC - 1),
                            )
                        if et % 2 == 0:
                            nc.scalar.copy(oo, ps_oo)
                        else:
                            nc.vector.tensor_copy(oo, ps_oo)
                        nc.sync.dma_start(
                            oT_b[:, et, sb * 512:(sb + 1) * 512], oo
                        )
                    return op
                return [mk(et) for et in range(KT)]

            # ---------------- S1: Q/K projections + RoPE ----------------
            for sb in range(SB):
                s1_qk_block(sb)
            # ------- S2: V projection, with qb=0 attention interleaved -------
            # Wo load (needed from qb=1 on; chunked so it spreads over queues)
            wo_sb = wbig.tile([128, HPC, HID], BF16, tag="w", name="wo")
            for sb in range(SB):
                s2_v_block(sb, wo_sb)
                if sb >= 1:
                    # head sb-1's qb=0 block: v rows 0..512 ready after sb=0
                    s3_attn_block(sb - 1, 0, [])
            s3_attn_block(SB - 1, 0, [])
            # ---------------- S3 + S4 interleaved ----------------
            for qb in range(1, SB):
                filler = oproj_ops(qb - 1)
                for h in range(HPC):
                    # h<2 flush (2..3, qb-1)'s normalizes (inside their own
                    # flush_z) BEFORE any filler reads attnT[:, sb=qb-1]
                    s3_attn_block(h, qb, filler if h >= 2 else [])
                for op in filler:
                    op()
            lag[0] = 0
            flush_z()
            for op in oproj_ops(SB - 1):
                op()
            if DEBUG:
                nc.sync.dma_start(d_rq.rearrange("p (h s) -> p h s", h=HPC), rq)
                nc.sync.dma_start(d_rk.rearrange("p (h s) -> p h s", h=HPC), rk)
                nc.sync.dma_start(d_v.rearrange("p (t j) -> p t j", t=ST), vnat)
                nc.sync.dma_start(d_at.rearrange("p (i j) -> p i j", i=HPC * SB), attnT)
                nc.sync.dma_start(d_z, zd)

    nc.compile()
    return nc


def _get_nc():
    if "fused" not in _CACHE:
        _CACHE["fused"] = build()
    return _CACHE["fused"]


def _causal_mask_templates():
    # masked (NEGM) iff 128*r + p > j for p in [0,128), j in [0,512)
    p = np.arange(128)[:, None]
    j = np.arange(512)[None, :]
    out = np.zeros((128, 4, 512), np.float32)
    for r in range(4):
        out[:, r, :] = np.where(128 * r + p > j, NEGM, 0.0).astype(np.float32)
    return np.ascontiguousarray(out.reshape(128, 4 * 512))


def _rope_cache_np():
    # mirrors reference._rope_cache in float32
    inv_freq = (1.0 / (BASE ** (np.arange(0, D, 2, dtype=np.float32) / np.float32(D)))).astype(np.float32)
    ratio = (MIN_R + (MAX_R - MIN_R) * (np.arange(H, dtype=np.float32) / np.float32(H))).astype(np.float32)
    t = (np.arange(S, dtype=np.float32)[None, :] / ratio[:, None]).astype(np.float32)
    freqs = (t[:, :, None] * inv_freq[None, None, :]).astype(np.float32)
    emb = np.concatenate([freqs, freqs], axis=-1)
    return np.cos(emb).astype(np.float32), np.sin(emb).astype(np.float32)


def _head_order_host(hs, Wq, Wk):
    """Exact Ms-PoE head ordering from the last-row attention stats.

    Only the last query row of the first softmax matters:
      srow[h] = q_last_h . K_h^T = hs @ (Wk_h^T q_last_h)
    computed here in fp64 (margins vs the fp32 reference are ~3e-5).
    """
    hs64 = hs.astype(np.float64)                      # [S, HID]
    q_last = Wq.astype(np.float64) @ hs64[-1]         # [HID]
    w = np.empty((HID, H), np.float64)
    for h in range(H):
        rows = slice(h * D, (h + 1) * D)
        w[:, h] = Wk[rows, :].astype(np.float64).T @ q_last[rows]
    srow = (hs64 @ w).T                               # [H, S]
    sc = srow * SCALE
    m = sc.max(axis=-1, keepdims=True)
    e = np.exp(sc - m)
    aw = e / e.sum(axis=-1, keepdims=True)
    avg = aw.mean(axis=-1, keepdims=True)
    cnt = (aw > 3.0 * avg).sum(axis=-1)
    outlier = (-(cnt / np.float32(S))).astype(np.float32)
    return np.argsort(outlier, kind="stable")


def kernel(hidden_states, position_ids, Wq, Wk, Wv, Wo):
    hs = np.asarray(hidden_states, dtype=np.float32)[0]        # [S, HID]
    pos = np.asarray(position_ids).astype(np.int64)[0]         # [S]
    Wq = np.asarray(Wq, dtype=np.float32)
    Wk = np.asarray(Wk, dtype=np.float32)
    Wv = np.asarray(Wv, dtype=np.float32)
    Wo = np.asarray(Wo, dtype=np.float32)

    # ---- host: head order + permuted RoPE caches ----
    head_order = _head_order_host(hs, Wq, Wk)
    cos, sin = _rope_cache_np()
    cos_o = cos[head_order][:, pos, :]                         # [H, S, D]
    sin_o = sin[head_order][:, pos, :]
    masks = _causal_mask_templates()

    hsT_bf = np.ascontiguousarray(hs.T).astype(BF_NP)          # [HID, S]

    nc = _get_nc()
    in_maps = []
    for c in range(NCORES):
        rows = slice(c * JC, (c + 1) * JC)
        ct = np.ascontiguousarray(
            np.concatenate([cos_o[c * HPC + i].T for i in range(HPC)], axis=0)
        )  # [JC, S]
        st = np.concatenate([sin_o[c * HPC + i].T for i in range(HPC)], axis=0)
        st = st.copy()
        for i in range(HPC):
            st[i * D: i * D + D // 2, :] *= -1.0
        in_maps.append(
            {
                "hsT": hsT_bf,
                "wqT": np.ascontiguousarray(Wq[rows, :].T).astype(BF_NP),
                "wkT": np.ascontiguousarray(Wk[rows, :].T).astype(BF_NP),
                "wvT": np.ascontiguousarray(Wv[rows, :].T).astype(BF_NP),
                "woT": np.ascontiguousarray(Wo[:, rows].T).astype(BF_NP),
                "cosT": ct.astype(BF_NP),
                "shatT": np.ascontiguousarray(st).astype(BF_NP),
                "masks": masks.astype(BF_NP),
            }
        )
    res = bass_utils.run_bass_kernel_spmd(
        nc, in_maps, core_ids=list(range(NCORES)), trace=TRACE
    )
    if TRACE:
        LAST_PROFILE["F"] = res
    global _LAST_RES
    _LAST_RES = res

    # ---- host: unshard (sum o_proj partials) ----
    acc = np.zeros((HID, S), np.float64)
    for c in range(NCORES):
        acc += res.results[c]["oT"].astype(np.float64)
    return np.ascontiguousarray(acc.T)[None, :, :].astype(np.float32)


# revision 21
# speedup vs baseline: 1.1781x; 1.1781x over previous
"""MsPoE Llama attention on 8 TRN2 NeuronCores (tensor-parallel over heads).

Strategy (v2, fused single launch)
----------------------------------
The Ms-PoE head ordering depends only on the LAST row of the first
softmax, which depends only on hs/Wq/Wk — all known on the host before
any launch. So the stats path runs on the HOST (fp64, margins ~3e-5 vs
the reference's fp32), and the device does ONE fused launch per core:

  per core (4 heads):
    S1: Q/K projections (bf16, W-stationary) -> per-head RoPE fused into
        the PSUM drain path (DVE + swap-half SBUF DMAs)
    S2: V projection (hs-stationary -> natural [s, d] layout)
    S3: causal attention per (head, 512-query block): scoresT = k^T q per
        128-key tile, exp on ACT (bf16 out), Z via elementwise ext
        accumulation (DVE/Pool) + one ones-matmul per block,
        normalization fused in-flight via 1/Z DRAM-broadcast roundtrip
    S4: o_proj partial oT[e, s] (bf16 out)

  host: argsort stats, permuted per-head RoPE cos/sin caches (bf16),
        final 8-way partial sum in fp64.

All matmuls are bf16 (PE full rate); PSUM accumulates fp32. Total
device rel err ~5e-3 against the fp32 reference, well under the 2e-2
gate. q/k/v never round-trip through HBM (SBUF-resident bf16).
"""

import os
import sys

import numpy as np

for _p in ("/opt/trn_rl_repo", "/root/.axon_site/_ro/trn_rl_repo"):
    if os.path.isdir(_p) and _p not in sys.path:
        sys.path.append(_p)

import ml_dtypes  # noqa: E402

import concourse.bass as bass  # noqa: E402
import concourse.tile as tile  # noqa: E402
from concourse import bacc, mybir  # noqa: E402
from concourse import bass_utils  # noqa: E402

F32 = mybir.dt.float32
F32R = mybir.dt.float32r
BF16 = mybir.dt.bfloat16
BF_NP = ml_dtypes.bfloat16

B, S, HID, H, D = 1, 2048, 4096, 32, 128
NCORES, HPC = 8, 4          # cores, heads per core
JC = HPC * D                # 512: per-core projection width
KT = HID // 128             # 32 contraction tiles
SB = S // 512               # 4 sequence blocks
ST = S // 128               # 16 key 128-tiles
BASE, MIN_R, MAX_R = 10000.0, 1.0, 3.0
SCALE = 1.0 / float(np.sqrt(D))
NEGM = -1.0e35              # additive causal mask value (exp -> 0)

_CACHE = {}
_LAST_RES = None
DEBUG = False
TRACE = False          # set True (e.g. from test.py) to profile the launch
LAST_PROFILE = {}      # filled with BassKernelResults when TRACE is on


def build():
    nc = bacc.Bacc("TRN2", target_bir_lowering=False, debug=False, num_devices=NCORES)
    hsT = nc.dram_tensor("hsT", [HID, S], BF16, kind="ExternalInput").ap()
    wqT = nc.dram_tensor("wqT", [HID, JC], BF16, kind="ExternalInput").ap()
    wkT = nc.dram_tensor("wkT", [HID, JC], BF16, kind="ExternalInput").ap()
    wvT = nc.dram_tensor("wvT", [HID, JC], BF16, kind="ExternalInput").ap()
    woT = nc.dram_tensor("woT", [JC, HID], BF16, kind="ExternalInput").ap()
    cosT = nc.dram_tensor("cosT", [JC, S], BF16, kind="ExternalInput").ap()
    shatT = nc.dram_tensor("shatT", [JC, S], BF16, kind="ExternalInput").ap()
    masks = nc.dram_tensor("masks", [128, 4 * 512], BF16, kind="ExternalInput").ap()
    oT = nc.dram_tensor("oT", [HID, S], BF16, kind="ExternalOutput").ap()
    if DEBUG:
        d_rq = nc.dram_tensor("d_rq", [128, HPC * S], BF16, kind="ExternalOutput").ap()
        d_rk = nc.dram_tensor("d_rk", [128, HPC * S], BF16, kind="ExternalOutput").ap()
        d_v = nc.dram_tensor("d_v", [128, ST * JC], BF16, kind="ExternalOutput").ap()
        d_at = nc.dram_tensor("d_at", [128, HPC * SB * 512], BF16, kind="ExternalOutput").ap()
        d_z = nc.dram_tensor("d_z", [HPC * SB, 512], F32, kind="ExternalOutput").ap()

    hsT_b = hsT.rearrange("(kt p) s -> p kt s", p=128)      # [128, 32, S]
    wqT_b = wqT.rearrange("(kt p) j -> p kt j", p=128)      # [128, 32, JC]
    wkT_b = wkT.rearrange("(kt p) j -> p kt j", p=128)
    wvT_b = wvT.rearrange("(kt p) j -> p kt j", p=128)
    woT_b = woT.rearrange("(jt p) e -> p jt e", p=128)      # [128, 4, HID]
    cosT_b = cosT.rearrange("(h p) s -> p h s", p=128)      # [128, 4, S]
    shatT_b = shatT.rearrange("(h p) s -> p h s", p=128)
    oT_b = oT.rearrange("(et p) s -> p et s", p=128)        # [128, 32, S]

    with tile.TileContext(nc) as tc:
        with (
            tc.tile_pool(name="wbig", bufs=3) as wbig,        # Wq/Wk/Wv/Wo ring
            tc.tile_pool(name="hpool", bufs=7) as hpool,      # hs stream [128,512]
            tc.tile_pool(name="qkch", bufs=4) as qkch,        # raw q/k chunks
            tc.tile_pool(name="swp", bufs=2) as swp,          # swapped-half chunks
            tc.tile_pool(name="csld", bufs=3) as csld,        # cos/sin chunks
            tc.tile_pool(name="rtmp", bufs=2) as rtmp,        # rope temps
            tc.tile_pool(name="ropes", bufs=1) as ropes,      # rq/rk resident
            tc.tile_pool(name="vres", bufs=1) as vres,        # v natural resident
            tc.tile_pool(name="attn", bufs=1) as attnp,       # attnT resident
            tc.tile_pool(name="expp", bufs=3) as expp,        # exp tiles bf16
            tc.tile_pool(name="accp", bufs=3) as accp,        # Z accumulators f32r
            tc.tile_pool(name="zp", bufs=2) as zp,            # z rows / broadcasts
            tc.tile_pool(name="small", bufs=1) as small,
            tc.tile_pool(name="outp", bufs=2) as outp,        # o_proj staging
            tc.tile_pool(name="dramp", bufs=1, space="DRAM") as dramp,
            tc.tile_pool(name="ps", bufs=8, space="PSUM") as ps,
        ):
            # ---------------- constants / weight loads ----------------
            masks_sb = small.tile([128, 4, 512], BF16)
            onesf_c = small.tile([128, 1], F32)
            nc.vector.memset(onesf_c, 1.0)
            ones_col = small.tile([128, 1], F32R)
            nc.vector.tensor_copy(ones_col, onesf_c)

            wq_sb = wbig.tile([128, KT, JC], BF16, tag="w", name="wq")
            wk_sb = wbig.tile([128, KT, JC], BF16, tag="w", name="wk")
            wv_sb = wbig.tile([128, KT, JC], BF16, tag="w", name="wv")

            rq = ropes.tile([128, HPC, S], BF16, name="rq")
            rk = ropes.tile([128, HPC, S], BF16, name="rk")
            vnat = vres.tile([128, ST, JC], BF16, name="vnat")
            attnT = attnp.tile([128, HPC * SB, 512], BF16, name="attnT")
            zd = dramp.tile([HPC * SB, 512], F32)

            def rope_chunk(h, sb, qch, kch):
                """rq/rk[:, h, sb*512:...] from raw chunks qch/kch [128,512]."""
                ss = slice(sb * 512, (sb + 1) * 512)
                cch = csld.tile([128, 512], BF16, tag="cs", name=f"c{h}_{sb}")
                nc.sync.dma_start(cch, cosT_b[:, h, ss])
                sch = csld.tile([128, 512], BF16, tag="cs", name=f"s{h}_{sb}")
                nc.sync.dma_start(sch, shatT_b[:, h, ss])
                qsw = swp.tile([128, 512], BF16, tag="sw", name=f"qsw{h}_{sb}")
                nc.sync.dma_start(qsw[0:64, :], qch[64:128, :])
                nc.sync.dma_start(qsw[64:128, :], qch[0:64, :])
                ksw = swp.tile([128, 512], BF16, tag="sw", name=f"ksw{h}_{sb}")
                nc.sync.dma_start(ksw[0:64, :], kch[64:128, :])
                nc.sync.dma_start(ksw[64:128, :], kch[0:64, :])
                with nc.allow_low_precision(reason="rope bf16"):
                    for eng, dst, raw, sw in (
                        (nc.vector, rq, qch, qsw),
                        (nc.gpsimd, rk, kch, ksw),
                    ):
                        d = dst[:, h, ss]
                        t2 = rtmp.tile([128, 512], BF16, tag="rt")
                        eng.tensor_mul(d, raw, cch)
                        eng.tensor_mul(t2, sw, sch)
                        eng.tensor_add(d, d, t2)

            def s1_qk_block(sb):
                ss = slice(sb * 512, (sb + 1) * 512)
                ps_q = [ps.tile([128, 512], F32, tag="ps", name=f"pq{sb}_{j}")
                        for j in range(HPC)]
                ps_k = [ps.tile([128, 512], F32, tag="ps", name=f"pk{sb}_{j}")
                        for j in range(HPC)]
                for kt in range(KT):
                    if sb == 0:
                        k1 = slice(kt, kt + 1)
                        nc.sync.dma_start(wq_sb[:, k1], wqT_b[:, k1])
                        nc.sync.dma_start(wk_sb[:, k1], wkT_b[:, k1])
                    if sb in (1, 2) and kt % 8 == 0:
                        # paced wv load (0.5MB per 8 kt) hides in the stream
                        ck = (sb - 1) * 4 + kt // 8
                        c4 = slice(ck * 4, (ck + 1) * 4)
                        nc.sync.dma_start(wv_sb[:, c4], wvT_b[:, c4])
                    if sb == 3 and kt == 0:
                        nc.sync.dma_start(
                            masks_sb, masks.rearrange("p (r j) -> p r j", r=4)
                        )
                    hst = hpool.tile([128, 512], BF16, tag="h",
                                     name=f"h1_{sb}_{kt}")
                    nc.sync.dma_start(hst, hsT_b[:, kt, ss])
                    for jt in range(HPC):
                        js = slice(jt * 128, (jt + 1) * 128)
                        nc.tensor.matmul(
                            ps_q[jt], wq_sb[:, kt, js], hst,
                            start=(kt == 0), stop=(kt == KT - 1),
                        )
                        nc.tensor.matmul(
                            ps_k[jt], wk_sb[:, kt, js], hst,
                            start=(kt == 0), stop=(kt == KT - 1),
                        )
                qcs, kcs = [], []
                for jt in range(HPC):
                    qcj = qkch.tile([128, 512], BF16, tag="qc", name=f"qc{sb}_{jt}")
                    kcj = qkch.tile([128, 512], BF16, tag="kc", name=f"kc{sb}_{jt}")
                    nc.scalar.copy(qcj, ps_q[jt])
                    nc.vector.tensor_copy(kcj, ps_k[jt])
                    qcs.append(qcj)
                    kcs.append(kcj)
                for h in range(HPC):
                    rope_chunk(h, sb, qcs[h], kcs[h])

            def s2_v_block(sb, wo_sb):
                ss = slice(sb * 512, (sb + 1) * 512)
                ps_v = [ps.tile([128, 512], F32, tag="ps", name=f"pv{sb}_{t}")
                        for t in range(4)]
                for kt in range(KT):
                    if kt % 16 == 8:
                        # paced wo load (0.5MB per 16 kt) for the S3 filler
                        ck = sb * 2 + kt // 16
                        cw = slice(ck * 512, (ck + 1) * 512)
                        nc.sync.dma_start(wo_sb[:, :, cw], woT_b[:, :, cw])
                    hst = hpool.tile([128, 512], BF16, tag="h", name=f"h2_{sb}_{kt}")
                    nc.sync.dma_start(hst, hsT_b[:, kt, ss])
                    for t4 in range(4):
                        cs = slice(t4 * 128, (t4 + 1) * 128)
                        nc.tensor.matmul(
                            ps_v[t4], hst[:, cs], wv_sb[:, kt],
                            start=(kt == 0), stop=(kt == KT - 1),
                        )
                for t4 in range(4):
                    if t4 % 2 == 0:
                        nc.scalar.copy(vnat[:, sb * 4 + t4], ps_v[t4])
                    else:
                        nc.vector.tensor_copy(vnat[:, sb * 4 + t4], ps_v[t4])

            pending_z = []
            lag = [1]

            def flush_z():
                # deferred Z chain: the ones-matmul waits on the Pool ext-sum,
                # so it is issued AFTER the next block's matmuls to avoid
                # head-of-line blocking the in-order PE queue
                while len(pending_z) > lag[0]:
                    i16, acc = pending_z.pop(0)
                    ps_z = ps.tile([1, 512], F32, tag="ps", name=f"pz{i16}")
                    nc.tensor.matmul(ps_z, ones_col, acc, start=True, stop=True)
                    zr1 = zp.tile([1, 512], F32, tag="zr", name=f"zr{i16}")
                    nc.vector.reciprocal(zr1, ps_z)
                    nc.sync.dma_start(zd[i16:i16 + 1, :], zr1)
                    zb = zp.tile([128, 512], F32, tag="zb", name=f"zb{i16}",
                                 bufs=3)
                    nc.sync.dma_start(
                        zb, zd[i16:i16 + 1, :].to_broadcast((128, 512))
                    )
                    with nc.allow_low_precision(reason="attn normalize bf16"):
                        nc.gpsimd.tensor_mul(attnT[:, i16], attnT[:, i16], zb)

            def s3_attn_block(h, qb, filler):
                hs_ = slice(h * 128, (h + 1) * 128)
                nkt = 4 * qb + 4
                qs = slice(qb * 512, (qb + 1) * 512)
                i16 = h * SB + qb
                ps_o = ps.tile([128, 512], F32, tag="ps", name=f"po{i16}")
                acc = accp.tile([128, 512], F32R, tag="acc", name=f"acc{i16}")
                for kt in range(nkt):
                    ps_s = ps.tile([128, 512], F32, tag="ps",
                                   name=f"s{i16}_{kt}")
                    nc.tensor.matmul(
                        ps_s, rk[:, h, kt * 128:(kt + 1) * 128], rq[:, h, qs],
                        start=True, stop=True,
                    )
                    r = kt - 4 * qb
                    if r >= 0:
                        nc.vector.tensor_add(ps_s, ps_s, masks_sb[:, r])
                    ext = expp.tile([128, 512], BF16, tag="exp")
                    nc.scalar.activation(
                        ext, ps_s, mybir.ActivationFunctionType.Exp,
                        scale=SCALE,
                    )
                    with nc.allow_low_precision(reason="z accum f32r"):
                        if kt == 0:
                            nc.gpsimd.tensor_copy(acc, ext)
                        else:
                            nc.gpsimd.tensor_add(acc, acc, ext)
                    nc.tensor.matmul(
                        ps_o, vnat[:, kt, hs_], ext,
                        start=(kt == 0), stop=(kt == nkt - 1),
                    )
                    if filler:
                        filler.pop(0)()
                flush_z()
                with nc.allow_low_precision(reason="attn drain bf16"):
                    nc.vector.tensor_copy(attnT[:, i16], ps_o)
                pending_z.append((i16, acc))

            def oproj_ops(sb):
                # per-et closures: emitted one at a time as always-ready PE
                # filler between attention kt iterations
                def mk(et):
                    def op():
                        es = slice(et * 128, (et + 1) * 128)
                        oo = outp.tile([128, 512], BF16, tag="oo",
                                       name=f"oo{et}_{sb}")
                        ps_oo = ps.tile([128, 512], F32, tag="ps",
                                        name=f"poo{et}_{sb}")
                        for jt in range(HPC):
                            nc.tensor.matmul(
                                ps_oo, wo_sb[:, jt, es],
                                attnT[:, jt * SB + sb],
                                start=(jt == 0), stop=(jt == HPC - 1),
                            )
                        if et % 2 == 0:
                            nc.scalar.copy(oo, ps_oo)
                        else:
                            nc.vector.tensor_copy(oo, ps_oo)
                        nc.sync.dma_start(
                            oT_b[:, et, sb * 512:(sb + 1) * 512], oo
                        )
                    return op
                return [mk(et) for et in range(KT)]

            # ---------------- S1: Q/K projections + RoPE ----------------
            for sb in range(SB):
                s1_qk_block(sb)
            # ------- S2: V projection, with qb=0 attention interleaved -------
            # Wo load (needed from qb=1 on; chunked so it spreads over queues)
            wo_sb = wbig.tile([128, HPC, HID], BF16, tag="w", name="wo")
            for sb in range(SB):
                s2_v_block(sb, wo_sb)
                if sb >= 1:
                    # head sb-1's qb=0 block: v rows 0..512 ready after sb=0
                    s3_attn_block(sb - 1, 0, [])
            s3_attn_block(SB - 1, 0, [])
            # ---------------- S3 + S4 interleaved ----------------
            for qb in range(1, SB):
                filler = oproj_ops(qb - 1)
                for h in range(HPC):
                    # h<2 flush (2..3, qb-1)'s normalizes (inside their own
                    # flush_z) BEFORE any filler reads attnT[:, sb=qb-1]
                    s3_attn_block(h, qb, filler if h >= 2 else [])
                for op in filler:
                    op()
            lag[0] = 0
            flush_z()
            for op in oproj_ops(SB - 1):
                op()
            if DEBUG:
                nc.sync.dma_start(d_rq.rearrange("p (h s) -> p h s", h=HPC), rq)
                nc.sync.dma_start(d_rk.rearrange("p (h s) -> p h s", h=HPC), rk)
                nc.sync.dma_start(d_v.rearrange("p (t j) -> p t j", t=ST), vnat)
                nc.sync.dma_start(d_at.rearrange("p (i j) -> p i j", i=HPC * SB), attnT)
                nc.sync.dma_start(d_z, zd)

    nc.compile()
    return nc


def _get_nc():
    if "fused" not in _CACHE:
        _CACHE["fused"] = build()
    return _CACHE["fused"]


def _causal_mask_templates():
    # masked (NEGM) iff 128*r + p > j for p in [0,128), j in [0,512)
    p = np.arange(128)[:, None]
    j = np.arange(512)[None, :]
    out = np.zeros((128, 4, 512), np.float32)
    for r in range(4):
        out[:, r, :] = np.where(128 * r + p > j, NEGM, 0.0).astype(np.float32)
    return np.ascontiguousarray(out.reshape(128, 4 * 512))


def _rope_cache_np():
    # mirrors reference._rope_cache in float32
    inv_freq = (1.0 / (BASE ** (np.arange(0, D, 2, dtype=np.float32) / np.float32(D)))).astype(np.float32)
    ratio = (MIN_R + (MAX_R - MIN_R) * (np.arange(H, dtype=np.float32) / np.float32(H))).astype(np.float32)
    t = (np.arange(S, dtype=np.float32)[None, :] / ratio[:, None]).astype(np.float32)
    freqs = (t[:, :, None] * inv_freq[None, None, :]).astype(np.float32)
    emb = np.concatenate([freqs, freqs], axis=-1)
    return np.cos(emb).astype(np.float32), np.sin(emb).astype(np.float32)


def _head_order_host(hs, Wq, Wk):
    """Exact Ms-PoE head ordering from the last-row attention stats.

    Only the last query row of the first softmax matters:
      srow[h] = q_last_h . K_h^T = hs @ (Wk_h^T q_last_h)
    computed here in fp64 (margins vs the fp32 reference are ~3e-5).
    """
    hs64 = hs.astype(np.float64)                      # [S, HID]
    q_last = Wq.astype(np.float64) @ hs64[-1]         # [HID]
    w = np.empty((HID, H), np.float64)
    for h in range(H):
        rows = slice(h * D, (h + 1) * D)
        w[:, h] = Wk[rows, :].astype(np.float64).T @ q_last[rows]
    srow = (hs64 @ w).T                               # [H, S]
    sc = srow * SCALE
    m = sc.max(axis=-1, keepdims=True)
    e = np.exp(sc - m)
    aw = e / e.sum(axis=-1, keepdims=True)
    avg = aw.mean(axis=-1, keepdims=True)
    cnt = (aw > 3.0 * avg).sum(axis=-1)
    outlier = (-(cnt / np.float32(S))).astype(np.float32)
    return np.argsort(outlier, kind="stable")


def kernel(hidden_states, position_ids, Wq, Wk, Wv, Wo):
    hs = np.asarray(hidden_states, dtype=np.float32)[0]        # [S, HID]
    pos = np.asarray(position_ids).astype(np.int64)[0]         # [S]
    Wq = np.asarray(Wq, dtype=np.float32)
    Wk = np.asarray(Wk, dtype=np.float32)
    Wv = np.asarray(Wv, dtype=np.float32)
    Wo = np.asarray(Wo, dtype=np.float32)

    # ---- host: head order + permuted RoPE caches ----
    head_order = _head_order_host(hs, Wq, Wk)
    cos, sin = _rope_cache_np()
    cos_o = cos[head_order][:, pos, :]                         # [H, S, D]
    sin_o = sin[head_order][:, pos, :]
    masks = _causal_mask_templates()

    hsT_bf = np.ascontiguousarray(hs.T).astype(BF_NP)          # [HID, S]

    nc = _get_nc()
    in_maps = []
    for c in range(NCORES):
        rows = slice(c * JC, (c + 1) * JC)
        ct = np.ascontiguousarray(
            np.concatenate([cos_o[c * HPC + i].T for i in range(HPC)], axis=0)
        )  # [JC, S]
        st = np.concatenate([sin_o[c * HPC + i].T for i in range(HPC)], axis=0)
        st = st.copy()
        for i in range(HPC):
            st[i * D: i * D + D // 2, :] *= -1.0
        in_maps.append(
            {
                "hsT": hsT_bf,
                "wqT": np.ascontiguousarray(Wq[rows, :].T).astype(BF_NP),
                "wkT": np.ascontiguousarray(Wk[rows, :].T).astype(BF_NP),
                "wvT": np.ascontiguousarray(Wv[rows, :].T).astype(BF_NP),
                "woT": np.ascontiguousarray(Wo[:, rows].T).astype(BF_NP),
                "cosT": ct.astype(BF_NP),
                "shatT": np.ascontiguousarray(st).astype(BF_NP),
                "masks": masks.astype(BF_NP),
            }
        )
    res = bass_utils.run_bass_kernel_spmd(
        nc, in_maps, core_ids=list(range(NCORES)), trace=TRACE
    )
    if TRACE:
        LAST_PROFILE["F"] = res
    global _LAST_RES
    _LAST_RES = res

    # ---- host: unshard (sum o_proj partials) ----
    acc = np.zeros((HID, S), np.float64)
    for c in range(NCORES):
        acc += res.results[c]["oT"].astype(np.float64)
    return np.ascontiguousarray(acc.T)[None, :, :].astype(np.float32)
